# revision 1
# baseline (speedup 1.0000x reference)
"""Trainium2 Bass kernel for nn_CoreDecoderStatefull (single-step stateful decoder).

Structure: dense -> 5x [GRU cell -> GLU -> concat -> stateful conv1d(k=2) -> concat]
-> output projection.  batch=1, seq=1: every matmul is a vector-matrix product.

Strategy (sharding hint: not shardable -> replicate on all 8 cores, read core 0):
  * All vectors live in SBUF as columns [<=128 partitions, 1]; every PE matmul
    is W.T-stationary with an x-column as the 1-wide moving operand (no
    transposes anywhere).
  * fp32 stationary weight load costs ~3.9 ns/column on TRN2 (measured), so
    recurrent-path matmuls (gi/glu/conv-x/out) run as THREE bf16 passes
    (Whi@xhi + Whi@xlo + Wlo@xhi, fp32 PSUM accumulate): ~2.3 ns/col, final
    rel-err ~8e-6 (validated against an fp64 host emulation).
  * Input-only mat-vecs (GRU h-terms, conv c-state taps, dense) run on the
    DVE as fp32 tensor_tensor_reduce (weights in natural [M,K] layout, input
    broadcast as a row, bias folded into the reduction init) -- early, under
    the weight-DMA shadow, keeping the PE free.
  * The concat vector x is stored as 6 chunk-columns of [128,6] bf16 hi/lo
    tiles: chunk c rows 0:96 = x0|g_c, rows 96:128 = conv out cv_{c+1};
    weight rows are permuted host-side to match.
  * PSUM group discipline: one start=True on the first matmul into a bank,
    one stop=True on the last; everything between start=False (first write
    to each byte range overwrites via the bank's pending-zero, then
    accumulates) -- this legalizes interleaving per-column groups.
  * PE work for stage s whose x-chunks completed at stage s-1 (full chunks of
    gi/conv-x/out) is emitted one stage early so only ~9 matmuls sit on the
    per-stage critical path.
  * Noise sites are deterministic (jax fold_in(key(42), i)) -> precomputed.
"""

import numpy as np
from contextlib import ExitStack

GD = [96, 224, 352, 480, 608]   # GRU input dims per stage
CD = [192, 320, 448, 576, 704]  # conv input dims per stage
N_CORES = 8


def _bf16(a):
    a = np.ascontiguousarray(np.asarray(a, np.float32))
    u = a.view(np.uint32)
    r = ((u + 0x7FFF + ((u >> 16) & 1)) & 0xFFFF0000).astype(np.uint32)
    return r.view(np.float32)


def _hl(a):
    hi = _bf16(a)
    return hi, _bf16(np.asarray(a, np.float32) - hi)


# ---------------------------------------------------------------------------
# reference x-vector index mapping
# ---------------------------------------------------------------------------
def _refidx(c: int, r: int) -> int:
    if r < 96:
        return r if c == 0 else GD[c - 1] + r      # x0 or g_c
    assert c <= 4
    return CD[c] + (r - 96)                        # cv_{c+1}


def _gi_chunks(s):
    return [(c, 128) for c in range(s - 1)] + [(s - 1, 96)]


def _cvx_chunks(s):
    return [(c, 128) for c in range(s - 1)] + [(s - 1, 96), (s, 96)]


_OUT_CHUNKS = [(c, 128) for c in range(5)] + [(5, 97)]  # row 96 = 1.0 (b_out)


# ---------------------------------------------------------------------------
# static layout
# ---------------------------------------------------------------------------
def _layout():
    # bf16 PE-weight slabs (slabs 1..5 = stage chains, 6 = out); each logical
    # block occupies 2*ncols bf16 columns: [hi | lo].
    wt = {}
    slab_cols = [0] * 7

    def put(name, slab, rows, ncols):
        wt[name] = (slab, slab_cols[slab], rows, ncols)
        slab_cols[slab] += 2 * ncols

    for s in range(1, 6):
        for (c, rows) in _gi_chunks(s):
            for j in range(3):
                put(f"gi{s}_{c}_{j}", s, rows, 96)
        put(f"glu{s}", s, 96, 96)
        for (c, rows) in _cvx_chunks(s):
            put(f"cvx{s}_{c}", s, rows, 32)
    for (c, rows) in _OUT_CHUNKS:
        put(f"out{c}", 6, rows, 80)

    # slab 0: early input-only matmuls (dense, gh, conv c-taps), hi/lo bf16,
    # in consumption order so the first DMA slab unblocks them fastest.
    put("dense", 0, 81, 96)
    for s in range(1, 6):
        for j in range(3):
            put(f"gh{s}_{j}", 0, 97, 96)
        cd = CD[s - 1]
        for j in range((cd + 127) // 128):
            put(f"cvc{s}_{j}", 0, min(128, cd - 128 * j), 32)

    # stile: fp32 column vectors
    st = {}
    scol = 0

    def sput(name, rows):
        nonlocal scol
        st[name] = (scol, rows)
        scol += 1

    sput("n0", 96)
    for s in range(1, 6):
        sput(f"nh{s}", 96)
        sput(f"ng{s}", 96)
        sput(f"ncv{s}", 32)
    for s in range(1, 6):
        sput(f"bin{s}", 96)            # bi[192:288] (gi n-gate bias, ACT bias)
        sput(f"cb{s}", 32)             # conv bias (cpart add)
        sput(f"hc{s}", 96)             # h state as fp32 column (GRU blend)
    # bf16 input columns (separate tile: matmul operands must be bf16);
    # aug 1.0 rows fold the biases: z row 80 = 1.0, h row 96 = 1.0
    sb = {}
    bcol = 0

    def bput(name, rows):
        nonlocal bcol
        sb[name] = (bcol, rows)
        bcol += 1

    for nm in ("zxh", "zxl"):
        bput(nm, 81)
    for s in range(1, 6):
        bput(f"hxh{s}", 97)
        bput(f"hxl{s}", 97)
    for s in range(1, 6):
        cd = CD[s - 1]
        for j in range((cd + 127) // 128):
            rows = min(128, cd - 128 * j)
            bput(f"cxh{s}_{j}", rows)
            bput(f"cxl{s}_{j}", rows)
    return wt, slab_cols, st, scol, sb, bcol


_WT, _SLAB_COLS, _ST, _ST_COLS, _SB, _SB_COLS = _layout()


# ---------------------------------------------------------------------------
# host-side packing
# ---------------------------------------------------------------------------
def _noise_vectors():
    import jax
    import jax.numpy as jnp

    vs = {}
    for i in range(16):
        n = 96 if (i == 0 or i % 3 != 0) else 32
        u = jax.random.uniform(
            jax.random.fold_in(jax.random.key(42), i), (1, n), dtype=jnp.float32
        )
        vs[i] = (np.asarray(u).reshape(-1) - 0.5) / np.float32(127.0)
    return vs


def _to_ml_bf16(a):
    import ml_dtypes

    return np.asarray(a, np.float32).astype(ml_dtypes.bfloat16)


def _pack(inp):
    f32 = np.float32
    slabs = [np.zeros((128, c), f32) for c in _SLAB_COLS]
    stile = np.zeros((128, _ST_COLS), f32)

    def wfill(name, block):
        slab, col, rows, ncols = _WT[name]
        assert block.shape == (rows, ncols), (name, block.shape)
        hi, lo = _hl(block)
        slabs[slab][:rows, col:col + ncols] = hi
        slabs[slab][:rows, col + ncols:col + 2 * ncols] = lo

    def sfill(name, vec):
        col, rows = _ST[name]
        assert vec.shape == (rows,), (name, vec.shape)
        stile[:rows, col] = vec

    sbf = np.zeros((128, _SB_COLS), f32)

    def sfill_hl(hname, lname, vec):
        col, rows = _SB[hname]
        hi, lo = _hl(vec)
        sbf[:rows, col] = hi
        col2, _ = _SB[lname]
        sbf[:rows, col2] = lo

    # early input-only PE weights (bias folded via aug-1 rows of the inputs)
    blk = np.zeros((81, 96), f32)
    blk[:80] = inp["w_dense"].T
    blk[80] = inp["b_dense"]
    wfill("dense", blk)
    for s in range(1, 6):
        wh, bi, bh = inp[f"g{s}_wh"], inp[f"g{s}_bi"], inp[f"g{s}_bh"]
        for j in range(3):
            blk = np.zeros((97, 96), f32)
            blk[:96] = wh[96 * j:96 * (j + 1), :].T
            blk[96] = (bi + bh)[96 * j:96 * (j + 1)] if j < 2 else bh[192:288]
            wfill(f"gh{s}_{j}", blk)
        cw0 = inp[f"cv{s}_w"][:, :, 0].astype(f32)
        cd = CD[s - 1]
        for j in range((cd + 127) // 128):
            rows = min(128, cd - 128 * j)
            wfill(f"cvc{s}_{j}", cw0[:, 128 * j:128 * j + rows].T)

    # PE (bf16 hi/lo) weights
    for s in range(1, 6):
        wi = inp[f"g{s}_wi"].astype(f32)
        for (c, rows) in _gi_chunks(s):
            ridx = [_refidx(c, r) for r in range(rows)]
            for j in range(3):
                wfill(f"gi{s}_{c}_{j}", wi[96 * j:96 * (j + 1), ridx].T)
        wfill(f"glu{s}", inp[f"glu{s}_w"].T.astype(f32))
        cw1 = inp[f"cv{s}_w"][:, :, 1].astype(f32)
        for (c, rows) in _cvx_chunks(s):
            ridx = [_refidx(c, r) for r in range(rows)]
            wfill(f"cvx{s}_{c}", cw1[:, ridx].T)
    w_out, b_out = inp["w_out"].astype(f32), inp["b_out"].astype(f32)
    for (c, rows) in _OUT_CHUNKS:
        if c < 5:
            ridx = [_refidx(c, r) for r in range(rows)]
            wfill(f"out{c}", w_out[:, ridx].T)
        else:
            blk = np.zeros((97, 80), f32)
            blk[:96] = w_out[:, 608:704].T
            blk[96] = b_out
            wfill(f"out{c}", blk)

    # state tile
    nv = _noise_vectors()
    sfill("n0", nv[0])
    for s in range(1, 6):
        sfill(f"nh{s}", nv[3 * s - 2])
        sfill(f"ng{s}", nv[3 * s - 1])
        sfill(f"ncv{s}", nv[3 * s])
        sfill(f"bin{s}", inp[f"g{s}_bi"][192:288].astype(f32))
        sfill(f"cb{s}", inp[f"cv{s}_b"].astype(f32))
        sfill(f"hc{s}", inp[f"h{s}"].reshape(-1).astype(f32))

    # hi/lo input columns for the early PE matmuls (aug row = 1.0)
    zv = np.zeros(81, f32)
    zv[:80] = inp["z"].reshape(-1)
    zv[80] = 1.0
    sfill_hl("zxh", "zxl", zv)
    for s in range(1, 6):
        hv = np.zeros(97, f32)
        hv[:96] = inp[f"h{s}"].reshape(-1)
        hv[96] = 1.0
        sfill_hl(f"hxh{s}", f"hxl{s}", hv)
        cv = inp[f"c{s}"].reshape(-1).astype(f32)
        cd = CD[s - 1]
        for j in range((cd + 127) // 128):
            rows = min(128, cd - 128 * j)
            sfill_hl(f"cxh{s}_{j}", f"cxl{s}_{j}", cv[128 * j:128 * j + rows])

    m = {f"wslab{i}": _to_ml_bf16(slabs[i]) for i in range(7)}
    m["stile"] = stile
    m["sbf16"] = _to_ml_bf16(sbf)
    return m


# ---------------------------------------------------------------------------
# device program
# ---------------------------------------------------------------------------
def _build_nc(loop_iters=None, dma_only=False, compute_only=False):
    from concourse import bacc, tile, mybir

    F32 = mybir.dt.float32
    BF16 = mybir.dt.bfloat16
    AF = mybir.ActivationFunctionType
    OP = mybir.AluOpType

    nc = bacc.Bacc("TRN2", target_bir_lowering=False, debug=False,
                   num_devices=N_CORES)
    wdram = {i: nc.dram_tensor(f"wslab{i}", [128, _SLAB_COLS[i]], BF16,
                               kind="ExternalInput") for i in range(7)}
    sdram = nc.dram_tensor("stile", [128, _ST_COLS], F32, kind="ExternalInput")
    bdram = nc.dram_tensor("sbf16", [128, _SB_COLS], BF16, kind="ExternalInput")
    ydram = nc.dram_tensor("y", [80, 1], F32, kind="ExternalOutput")

    with tile.TileContext(nc) as tc, ExitStack() as ctx:
        wpool = ctx.enter_context(tc.tile_pool(name="wpool", bufs=1))
        spool = ctx.enter_context(tc.tile_pool(name="spool", bufs=1))
        work = ctx.enter_context(tc.tile_pool(name="work", bufs=1))
        pearly = ctx.enter_context(tc.tile_pool(name="pearly", bufs=2, space="PSUM"))
        pgi = ctx.enter_context(tc.tile_pool(name="pgi", bufs=2, space="PSUM"))
        pglu = ctx.enter_context(tc.tile_pool(name="pglu", bufs=1, space="PSUM"))
        pconv = ctx.enter_context(tc.tile_pool(name="pconv", bufs=2, space="PSUM"))
        pout = ctx.enter_context(tc.tile_pool(name="pout", bufs=1, space="PSUM"))

        if loop_iters is not None:
            ctx.enter_context(tc.For_i(0, loop_iters, 1))

        # ACT table prefetch (sigmoid_and_others holds sigmoid+tanh)
        warm = work.tile([1, 1], F32, tag="warm", name="warm")
        nc.vector.memset(warm[:], 0.0)
        warm2 = work.tile([1, 1], F32, tag="warm2", name="warm2")
        nc.scalar.activation(warm2[:], warm[:], AF.Sigmoid)
        nc.scalar.activation(warm2[:], warm2[:], AF.Tanh)

        XH = work.tile([128, 6], BF16, tag="XH", name="XH")
        XL = work.tile([128, 6], BF16, tag="XL", name="XL")
        nc.vector.memset(XH[96:97, 5:6], 1.0)
        nc.vector.memset(XL[96:97, 5:6], 0.0)

        stile = spool.tile([128, _ST_COLS], F32, tag="stile", name="stile")
        sbf = spool.tile([128, _SB_COLS], BF16, tag="sbf", name="sbf")
        wt = {}
        if not compute_only:
            nc.sync.dma_start(out=stile[:], in_=sdram[:])
            nc.sync.dma_start(out=sbf[:], in_=bdram[:])
        for i in range(7):
            t = wpool.tile([128, _SLAB_COLS[i]], BF16, tag=f"w{i}", name=f"wt{i}")
            if not compute_only:
                nc.sync.dma_start(out=t[:], in_=wdram[i][:])
            wt[i] = t
        if compute_only:
            nc.vector.memset(stile[:, 0:1], 0.01)
            nc.vector.memset(sbf[:, 0:1], 0.01)
            for i in range(7):
                nc.vector.memset(wt[i][:, 0:1], 0.01)

        def WH(name):
            slab, col, rows, ncols = _WT[name]
            return wt[slab][0:rows, col:col + ncols]

        def WL(name):
            slab, col, rows, ncols = _WT[name]
            return wt[slab][0:rows, col + ncols:col + 2 * ncols]

        def S(name, rows=None):
            col, r = _ST[name]
            if rows is not None:
                r = rows
            return stile[0:r, col:col + 1]

        def SB(name):
            col, r = _SB[name]
            return sbf[0:r, col:col + 1]

        def mm3h(psum_ap, name, xhi, xlo, start, stop):
            # 3-pass bf16 hi/lo matmul accumulate into psum_ap
            nc.tensor.matmul(psum_ap, WH(name), xhi, start=start, stop=False)
            nc.tensor.matmul(psum_ap, WH(name), xlo, start=False, stop=False)
            nc.tensor.matmul(psum_ap, WL(name), xhi, start=False, stop=stop)

        def mm3(psum_ap, name, xc, rows, start, stop):
            mm3h(psum_ap, name, XH[0:rows, xc:xc + 1], XL[0:rows, xc:xc + 1],
                 start, stop)

        if not dma_only:
            # ---- early input-only matmuls, under the DMA shadow ----
            pd = pearly.tile([96, 3], F32, tag="early", name="pdense")
            mm3h(pd[:96, 0:1], "dense", SB("zxh"), SB("zxl"), True, True)
            x0t = work.tile([96, 1], F32, tag="x0t", name="x0t")
            nc.scalar.activation(x0t[:], pd[:96, 0:1], AF.Tanh)
            tx0 = work.tile([96, 1], F32, tag="tx0", name="tx0")
            nc.vector.tensor_scalar(tx0[:], x0t[:], S("n0"), -1.0, OP.add, OP.max)
            x0f = work.tile([96, 1], F32, tag="x0f", name="x0f")
            nc.vector.tensor_scalar(x0f[:], tx0[:], 1.0, None, OP.min)
            nc.vector.tensor_copy(XH[0:96, 0:1], x0f[:])
            nc.vector.tensor_tensor(XL[0:96, 0:1], x0f[:], XH[0:96, 0:1],
                                    OP.subtract)

            ghsb, cpart = {}, {}
            for s in range(1, 6):
                pg = pearly.tile([96, 3], F32, tag="early", name=f"pgh{s}")
                for ji, j in enumerate(range(3)):
                    mm3h(pg[:, j:j + 1], f"gh{s}_{j}", SB(f"hxh{s}"),
                         SB(f"hxl{s}"), start=(ji == 0), stop=(ji == 2))
                g = work.tile([96, 3], F32, tag=f"ghsb{s}", name=f"ghsb{s}")
                nc.vector.tensor_copy(g[:], pg[:])
                ghsb[s] = g
                pc = pearly.tile([96, 3], F32, tag="early", name=f"pcvc{s}")
                nch = (CD[s - 1] + 127) // 128
                for j in range(nch):
                    mm3h(pc[0:32, 0:1], f"cvc{s}_{j}", SB(f"cxh{s}_{j}"),
                         SB(f"cxl{s}_{j}"), start=(j == 0), stop=(j == nch - 1))
                cp = work.tile([32, 1], F32, tag=f"cpart{s}", name=f"cpart{s}")
                nc.vector.tensor_scalar(cp[:], pc[0:32, 0:1], S(f"cb{s}"), None,
                                        OP.add)
                cpart[s] = cp

            # ---- sequential chain with one-stage-ahead PE emission ----
            P, R = {}, {}
            O = pout.tile([80, 1], F32, tag="out", name="Oout")

            for s in range(1, 6):
                if s >= 2:
                    # eager: stage-s PE tiles whose x-chunks are complete
                    P[s] = pgi.tile([96, 3], F32, tag="gi", name=f"Pgi{s}")
                    chunks = _gi_chunks(s)
                    first = True
                    for j in (0, 2, 1):
                        for (c, rows) in chunks[:-1]:
                            mm3(P[s][:, j:j + 1], f"gi{s}_{c}_{j}", c, rows,
                                start=first, stop=False)
                            first = False
                    R[s] = pconv.tile([32, 1], F32, tag="cv", name=f"R{s}")
                    cchunks = _cvx_chunks(s)
                    first = True
                    for (c, rows) in cchunks[:-2]:
                        mm3(R[s][:], f"cvx{s}_{c}", c, rows,
                            start=first, stop=False)
                        first = False
                    c, rows = _OUT_CHUNKS[s - 2]
                    mm3(O[:], f"out{c}", c, rows, start=(s == 2), stop=False)
                else:
                    P[1] = pgi.tile([96, 3], F32, tag="gi", name="Pgi1")
                    R[1] = pconv.tile([32, 1], F32, tag="cv", name="R1")

                Ps, Rs = P[s], R[s]
                c_last, rows_last = _gi_chunks(s)[-1]
                for ji, j in enumerate((0, 2, 1)):
                    mm3(Ps[:, j:j + 1], f"gi{s}_{c_last}_{j}", c_last, rows_last,
                        start=(s == 1 and ji == 0), stop=(ji == 2))

                r = work.tile([96, 1], F32, tag="r", name=f"r{s}")
                z = work.tile([96, 1], F32, tag="zz", name=f"z{s}")
                nc.scalar.activation(r[:], Ps[:, 0:1], AF.Sigmoid,
                                     bias=ghsb[s][:, 0:1])
                nc.scalar.activation(z[:], Ps[:, 1:2], AF.Sigmoid,
                                     bias=ghsb[s][:, 1:2])
                t2 = work.tile([96, 1], F32, tag="t2", name=f"t2_{s}")
                # t2 = gh_n * r + gi_n
                nc.vector.scalar_tensor_tensor(t2[:], ghsb[s][:, 2:3], r[:],
                                               Ps[:, 2:3], OP.mult, OP.add)
                c_ = work.tile([96, 1], F32, tag="c_", name=f"c{s}_")
                nc.scalar.activation(c_[:], t2[:], AF.Tanh, bias=S(f"bin{s}"))
                cn = work.tile([96, 1], F32, tag="cn", name=f"cn{s}")
                nc.vector.tensor_add(cn[:], c_[:], S(f"nh{s}"))
                hnp = work.tile([96, 1], F32, tag="hnp", name=f"hnp{s}")
                # hnp = (h - c) * z
                nc.vector.scalar_tensor_tensor(hnp[:], S(f"hc{s}"), c_[:], z[:],
                                               OP.subtract, OP.mult)
                t3 = work.tile([96, 1], F32, tag="t3", name=f"t3_{s}")
                nc.vector.tensor_scalar(t3[:], hnp[:], cn[:], -1.0, OP.add, OP.max)
                hn3 = work.tile([96, 1], F32, tag="hn3", name=f"hn3_{s}")
                nc.vector.tensor_scalar(hn3[:], t3[:], 1.0, None, OP.min)
                hnh = work.tile([96, 1], BF16, tag="hnh", name=f"hnh{s}")
                nc.vector.tensor_copy(hnh[:], hn3[:])
                hnl = work.tile([96, 1], BF16, tag="hnl", name=f"hnl{s}")
                nc.vector.tensor_tensor(hnl[:], hn3[:], hnh[:], OP.subtract)

                Q = pglu.tile([96, 1], F32, tag="glu", name=f"Q{s}")
                nc.tensor.matmul(Q[:], WH(f"glu{s}"), hnh[:], start=True, stop=False)
                nc.tensor.matmul(Q[:], WH(f"glu{s}"), hnl[:], start=False, stop=False)
                nc.tensor.matmul(Q[:], WL(f"glu{s}"), hnh[:], start=False, stop=True)
                sg = work.tile([96, 1], F32, tag="sg", name=f"sg{s}")
                nc.scalar.activation(sg[:], Q[:], AF.Sigmoid)
                g0 = work.tile([96, 1], F32, tag="g0", name=f"g0_{s}")
                nc.vector.scalar_tensor_tensor(g0[:], sg[:], hn3[:], S(f"ng{s}"),
                                               OP.mult, OP.add)
                gf = work.tile([96, 1], F32, tag="gf", name=f"gf{s}")
                nc.vector.tensor_scalar(gf[:], g0[:], -1.0, 1.0, OP.max, OP.min)
                nc.vector.tensor_copy(XH[0:96, s:s + 1], gf[:])
                nc.vector.tensor_tensor(XL[0:96, s:s + 1], gf[:],
                                        XH[0:96, s:s + 1], OP.subtract)

                cchunks = _cvx_chunks(s)
                for idx, (c, rows) in enumerate(cchunks[-2:]):
                    mm3(Rs[:], f"cvx{s}_{c}", c, rows,
                        start=(s == 1 and idx == 0), stop=(idx == 1))
                cv = work.tile([32, 1], F32, tag="cv_", name=f"cv{s}_")
                nc.scalar.activation(cv[:], Rs[:], AF.Tanh, bias=cpart[s][:])
                cv0 = work.tile([32, 1], F32, tag="cv0", name=f"cv0_{s}")
                nc.vector.tensor_scalar(cv0[:], cv[:], S(f"ncv{s}"), -1.0,
                                        OP.add, OP.max)
                cvf = work.tile([32, 1], F32, tag="cvf", name=f"cvf{s}")
                nc.vector.tensor_scalar(cvf[:], cv0[:], 1.0, None, OP.min)
                # split at base partition 0, then move (tensor_scalar permits
                # differing start partitions; tensor_tensor does not)
                cvh0 = work.tile([32, 1], BF16, tag="cvh0", name=f"cvh0_{s}")
                nc.vector.tensor_copy(cvh0[:], cvf[:])
                cvl0 = work.tile([32, 1], BF16, tag="cvl0", name=f"cvl0_{s}")
                nc.vector.tensor_tensor(cvl0[:], cvf[:], cvh0[:], OP.subtract)
                nc.vector.tensor_scalar_add(XH[96:128, s - 1:s], cvh0[:], 0.0)
                nc.vector.tensor_scalar_add(XL[96:128, s - 1:s], cvl0[:], 0.0)

            # ---- output projection tail (chunks 4,5 need stage-5 outputs)
            for (c, rows) in _OUT_CHUNKS[4:]:
                mm3(O[:], f"out{c}", c, rows, start=False, stop=(c == 5))
            y_sb = work.tile([80, 1], F32, tag="y", name="y_sb")
            nc.vector.tensor_copy(y_sb[:], O[:])
            nc.sync.dma_start(out=ydram[:], in_=y_sb[:])

    nc.compile()
    return nc


_NC_CACHE = None


def _get_nc():
    global _NC_CACHE
    if _NC_CACHE is None:
        _NC_CACHE = _build_nc()
    return _NC_CACHE


def kernel(**inputs) -> np.ndarray:
    from concourse.bass_utils import run_bass_kernel_spmd

    nc = _get_nc()
    in_map = _pack(inputs)
    in_maps = [in_map for _ in range(N_CORES)]
    res = run_bass_kernel_spmd(nc, in_maps, list(range(N_CORES)))
    y = np.asarray(res.results[0]["y"]).reshape(-1)
    return y.reshape(1, 4, 20).astype(np.float32)



# revision 2
# speedup vs baseline: 1.6812x; 1.6812x over previous
"""Trainium2 Bass kernel for nn_CoreDecoderStatefull (single-step stateful decoder).

Structure: dense -> 5x [GRU cell -> GLU -> concat -> stateful conv1d(k=2) -> concat]
-> output projection.  batch=1, seq=1: every matmul is a vector-matrix product.

Strategy (sharding hint: not shardable -> replicate on all 8 cores, read core 0):
  * All vectors live in SBUF as columns [<=128 partitions, 1]; every PE matmul
    is W.T-stationary with an x-column as the 1-wide moving operand (no
    transposes anywhere).
  * Weights are single-copy bf16 (the correctness gate is 2e-2; measured
    ~3e-4).  PE cost is dominated by Ldweights at ~1 column/cycle, so
    single-pass bf16 is ~3x less PE time and ~2x less HBM traffic than the
    earlier hi/lo 3-pass scheme.
  * The concat vector x is stored as 6 chunk-columns of a [128,6] bf16 tile:
    chunk c rows 0:96 = x0|g_c, rows 96:128 = conv out cv_{c+1}; weight rows
    are permuted host-side to match.
  * PSUM group discipline: one start=True on the first matmul into a bank,
    one stop=True on the last; everything between start=False (first write
    to each byte range overwrites via the bank's pending-zero, then
    accumulates) -- this legalizes interleaving per-column groups.
  * PE work for stage s whose x-chunks completed at stage s-1 (full chunks of
    gi/conv-x/out) is emitted one stage early so only ~9 matmuls sit on the
    per-stage critical path.
  * Noise sites are deterministic (jax fold_in(key(42), i)) -> precomputed.
"""

import numpy as np
from contextlib import ExitStack

GD = [96, 224, 352, 480, 608]   # GRU input dims per stage
CD = [192, 320, 448, 576, 704]  # conv input dims per stage
N_CORES = 8


# ---------------------------------------------------------------------------
# reference x-vector index mapping
# ---------------------------------------------------------------------------
def _refidx(c: int, r: int) -> int:
    if r < 96:
        return r if c == 0 else GD[c - 1] + r      # x0 or g_c
    assert c <= 4
    return CD[c] + (r - 96)                        # cv_{c+1}


def _gi_chunks(s):
    return [(c, 128) for c in range(s - 1)] + [(s - 1, 96)]


def _cvx_chunks(s):
    return [(c, 128) for c in range(s - 1)] + [(s - 1, 96), (s, 96)]


_OUT_CHUNKS = [(c, 128) for c in range(5)] + [(5, 97)]  # row 96 = 1.0 (b_out)


# ---------------------------------------------------------------------------
# static layout
# ---------------------------------------------------------------------------
def _layout():
    # bf16 PE-weight slabs (slabs 1..5 = stage chains, 6 = out).
    wt = {}
    slab_cols = [0] * 7

    def put(name, slab, rows, ncols):
        wt[name] = (slab, slab_cols[slab], rows, ncols)
        slab_cols[slab] += ncols

    for s in range(1, 6):
        for (c, rows) in _gi_chunks(s):
            for j in range(3):
                put(f"gi{s}_{c}_{j}", s, rows, 96)
        put(f"glu{s}", s, 96, 96)
        for (c, rows) in _cvx_chunks(s):
            put(f"cvx{s}_{c}", s, rows, 32)
    for (c, rows) in _OUT_CHUNKS:
        put(f"out{c}", 6, rows, 80)

    # slab 0: early input-only matmuls (dense, gh, conv c-taps), in
    # consumption order so the first DMA slab unblocks them fastest.
    put("dense", 0, 81, 96)
    for s in range(1, 6):
        for j in range(3):
            put(f"gh{s}_{j}", 0, 97, 96)
        cd = CD[s - 1]
        for j in range((cd + 127) // 128):
            put(f"cvc{s}_{j}", 0, min(128, cd - 128 * j), 32)

    # stile: fp32 column vectors
    st = {}
    scol = 0

    def sput(name, rows):
        nonlocal scol
        st[name] = (scol, rows)
        scol += 1

    sput("n0", 96)
    for s in range(1, 6):
        sput(f"nh{s}", 96)
        sput(f"ng{s}", 96)
        sput(f"ncv{s}", 32)
    for s in range(1, 6):
        sput(f"bin{s}", 96)            # bi[192:288] (gi n-gate bias, ACT bias)
        sput(f"cb{s}", 32)             # conv bias (cpart add)
        sput(f"hc{s}", 96)             # h state as fp32 column (GRU blend)
    # bf16 input columns (matmul operands must be bf16);
    # aug 1.0 rows fold the biases: z row 80 = 1.0, h row 96 = 1.0
    sb = {}
    bcol = 0

    def bput(name, rows):
        nonlocal bcol
        sb[name] = (bcol, rows)
        bcol += 1

    bput("zx", 81)
    for s in range(1, 6):
        bput(f"hx{s}", 97)
    for s in range(1, 6):
        cd = CD[s - 1]
        for j in range((cd + 127) // 128):
            bput(f"cx{s}_{j}", min(128, cd - 128 * j))
    return wt, slab_cols, st, scol, sb, bcol


_WT, _SLAB_COLS, _ST, _ST_COLS, _SB, _SB_COLS = _layout()


# ---------------------------------------------------------------------------
# host-side packing
# ---------------------------------------------------------------------------
def _noise_vectors():
    import jax
    import jax.numpy as jnp

    vs = {}
    for i in range(16):
        n = 96 if (i == 0 or i % 3 != 0) else 32
        u = jax.random.uniform(
            jax.random.fold_in(jax.random.key(42), i), (1, n), dtype=jnp.float32
        )
        vs[i] = (np.asarray(u).reshape(-1) - 0.5) / np.float32(127.0)
    return vs


def _to_ml_bf16(a):
    import ml_dtypes

    return np.asarray(a, np.float32).astype(ml_dtypes.bfloat16)


def _pack(inp):
    f32 = np.float32
    slabs = [np.zeros((128, c), f32) for c in _SLAB_COLS]
    stile = np.zeros((128, _ST_COLS), f32)

    def wfill(name, block):
        slab, col, rows, ncols = _WT[name]
        assert block.shape == (rows, ncols), (name, block.shape)
        slabs[slab][:rows, col:col + ncols] = block

    def sfill(name, vec):
        col, rows = _ST[name]
        assert vec.shape == (rows,), (name, vec.shape)
        stile[:rows, col] = vec

    sbf = np.zeros((128, _SB_COLS), f32)

    def bfill(name, vec):
        col, rows = _SB[name]
        assert vec.shape == (rows,), (name, vec.shape)
        sbf[:rows, col] = vec

    # early input-only PE weights (bias folded via aug-1 rows of the inputs)
    blk = np.zeros((81, 96), f32)
    blk[:80] = inp["w_dense"].T
    blk[80] = inp["b_dense"]
    wfill("dense", blk)
    for s in range(1, 6):
        wh, bi, bh = inp[f"g{s}_wh"], inp[f"g{s}_bi"], inp[f"g{s}_bh"]
        for j in range(3):
            blk = np.zeros((97, 96), f32)
            blk[:96] = wh[96 * j:96 * (j + 1), :].T
            blk[96] = (bi + bh)[96 * j:96 * (j + 1)] if j < 2 else bh[192:288]
            wfill(f"gh{s}_{j}", blk)
        cw0 = inp[f"cv{s}_w"][:, :, 0].astype(f32)
        cd = CD[s - 1]
        for j in range((cd + 127) // 128):
            rows = min(128, cd - 128 * j)
            wfill(f"cvc{s}_{j}", cw0[:, 128 * j:128 * j + rows].T)

    # recurrent-path PE weights
    for s in range(1, 6):
        wi = inp[f"g{s}_wi"].astype(f32)
        for (c, rows) in _gi_chunks(s):
            ridx = [_refidx(c, r) for r in range(rows)]
            for j in range(3):
                wfill(f"gi{s}_{c}_{j}", wi[96 * j:96 * (j + 1), ridx].T)
        wfill(f"glu{s}", inp[f"glu{s}_w"].T.astype(f32))
        cw1 = inp[f"cv{s}_w"][:, :, 1].astype(f32)
        for (c, rows) in _cvx_chunks(s):
            ridx = [_refidx(c, r) for r in range(rows)]
            wfill(f"cvx{s}_{c}", cw1[:, ridx].T)
    w_out, b_out = inp["w_out"].astype(f32), inp["b_out"].astype(f32)
    for (c, rows) in _OUT_CHUNKS:
        if c < 5:
            ridx = [_refidx(c, r) for r in range(rows)]
            wfill(f"out{c}", w_out[:, ridx].T)
        else:
            blk = np.zeros((97, 80), f32)
            blk[:96] = w_out[:, 608:704].T
            blk[96] = b_out
            wfill(f"out{c}", blk)

    # state tile
    nv = _noise_vectors()
    sfill("n0", nv[0])
    for s in range(1, 6):
        sfill(f"nh{s}", nv[3 * s - 2])
        sfill(f"ng{s}", nv[3 * s - 1])
        sfill(f"ncv{s}", nv[3 * s])
        sfill(f"bin{s}", inp[f"g{s}_bi"][192:288].astype(f32))
        sfill(f"cb{s}", inp[f"cv{s}_b"].astype(f32))
        sfill(f"hc{s}", inp[f"h{s}"].reshape(-1).astype(f32))

    # bf16 input columns for the early PE matmuls (aug row = 1.0)
    zv = np.zeros(81, f32)
    zv[:80] = inp["z"].reshape(-1)
    zv[80] = 1.0
    bfill("zx", zv)
    for s in range(1, 6):
        hv = np.zeros(97, f32)
        hv[:96] = inp[f"h{s}"].reshape(-1)
        hv[96] = 1.0
        bfill(f"hx{s}", hv)
        cv = inp[f"c{s}"].reshape(-1).astype(f32)
        cd = CD[s - 1]
        for j in range((cd + 127) // 128):
            rows = min(128, cd - 128 * j)
            bfill(f"cx{s}_{j}", cv[128 * j:128 * j + rows])

    m = {f"wslab{i}": _to_ml_bf16(slabs[i]) for i in range(7)}
    m["stile"] = stile
    m["sbf16"] = _to_ml_bf16(sbf)
    return m


# ---------------------------------------------------------------------------
# device program
# ---------------------------------------------------------------------------
def _build_nc(loop_iters=None, dma_only=False, compute_only=False):
    from concourse import bacc, tile, mybir

    F32 = mybir.dt.float32
    BF16 = mybir.dt.bfloat16
    AF = mybir.ActivationFunctionType
    OP = mybir.AluOpType

    nc = bacc.Bacc("TRN2", target_bir_lowering=False, debug=False,
                   num_devices=N_CORES)
    wdram = {i: nc.dram_tensor(f"wslab{i}", [128, _SLAB_COLS[i]], BF16,
                               kind="ExternalInput") for i in range(7)}
    sdram = nc.dram_tensor("stile", [128, _ST_COLS], F32, kind="ExternalInput")
    bdram = nc.dram_tensor("sbf16", [128, _SB_COLS], BF16, kind="ExternalInput")
    ydram = nc.dram_tensor("y", [80, 1], F32, kind="ExternalOutput")

    with tile.TileContext(nc) as tc, ExitStack() as ctx:
        wpool = ctx.enter_context(tc.tile_pool(name="wpool", bufs=1))
        spool = ctx.enter_context(tc.tile_pool(name="spool", bufs=1))
        work = ctx.enter_context(tc.tile_pool(name="work", bufs=1))
        pearly = ctx.enter_context(tc.tile_pool(name="pearly", bufs=2, space="PSUM"))
        pgi = ctx.enter_context(tc.tile_pool(name="pgi", bufs=2, space="PSUM"))
        pglu = ctx.enter_context(tc.tile_pool(name="pglu", bufs=1, space="PSUM"))
        pconv = ctx.enter_context(tc.tile_pool(name="pconv", bufs=2, space="PSUM"))
        pout = ctx.enter_context(tc.tile_pool(name="pout", bufs=1, space="PSUM"))

        if loop_iters is not None:
            ctx.enter_context(tc.For_i(0, loop_iters, 1))

        # ACT table prefetch (sigmoid_and_others holds sigmoid+tanh)
        warm = work.tile([1, 1], F32, tag="warm", name="warm")
        nc.vector.memset(warm[:], 0.0)
        warm2 = work.tile([1, 1], F32, tag="warm2", name="warm2")
        nc.scalar.activation(warm2[:], warm[:], AF.Sigmoid)
        nc.scalar.activation(warm2[:], warm2[:], AF.Tanh)

        XH = work.tile([128, 6], BF16, tag="XH", name="XH")
        nc.vector.memset(XH[96:97, 5:6], 1.0)

        stile = spool.tile([128, _ST_COLS], F32, tag="stile", name="stile")
        sbf = spool.tile([128, _SB_COLS], BF16, tag="sbf", name="sbf")
        wt = {}
        if not compute_only:
            nc.sync.dma_start(out=stile[:], in_=sdram[:])
            nc.sync.dma_start(out=sbf[:], in_=bdram[:])
        for i in range(7):
            t = wpool.tile([128, _SLAB_COLS[i]], BF16, tag=f"w{i}", name=f"wt{i}")
            if not compute_only:
                nc.sync.dma_start(out=t[:], in_=wdram[i][:])
            wt[i] = t
        if compute_only:
            nc.vector.memset(stile[:, 0:1], 0.01)
            nc.vector.memset(sbf[:, 0:1], 0.01)
            for i in range(7):
                nc.vector.memset(wt[i][:, 0:1], 0.01)

        def WH(name):
            slab, col, rows, ncols = _WT[name]
            return wt[slab][0:rows, col:col + ncols]

        def S(name, rows=None):
            col, r = _ST[name]
            if rows is not None:
                r = rows
            return stile[0:r, col:col + 1]

        def SB(name):
            col, r = _SB[name]
            return sbf[0:r, col:col + 1]

        def mm(psum_ap, name, xc, rows, start, stop):
            nc.tensor.matmul(psum_ap, WH(name), XH[0:rows, xc:xc + 1],
                             start=start, stop=stop)

        if not dma_only:
            # ---- early input-only matmuls, under the DMA shadow ----
            pd = pearly.tile([96, 3], F32, tag="early", name="pdense")
            nc.tensor.matmul(pd[:96, 0:1], WH("dense"), SB("zx"),
                             start=True, stop=True)
            x0t = work.tile([96, 1], F32, tag="x0t", name="x0t")
            nc.scalar.activation(x0t[:], pd[:96, 0:1], AF.Tanh)
            tx0 = work.tile([96, 1], F32, tag="tx0", name="tx0")
            nc.vector.tensor_scalar(tx0[:], x0t[:], S("n0"), -1.0, OP.add, OP.max)
            x0f = work.tile([96, 1], F32, tag="x0f", name="x0f")
            nc.vector.tensor_scalar(x0f[:], tx0[:], 1.0, None, OP.min)
            nc.vector.tensor_copy(XH[0:96, 0:1], x0f[:])

            ghsb, cpart = {}, {}
            for s in range(1, 6):
                pg = pearly.tile([96, 3], F32, tag="early", name=f"pgh{s}")
                for j in range(3):
                    nc.tensor.matmul(pg[:, j:j + 1], WH(f"gh{s}_{j}"),
                                     SB(f"hx{s}"), start=(j == 0), stop=(j == 2))
                g = work.tile([96, 3], F32, tag=f"ghsb{s}", name=f"ghsb{s}")
                nc.vector.tensor_copy(g[:], pg[:])
                ghsb[s] = g
                pc = pearly.tile([96, 3], F32, tag="early", name=f"pcvc{s}")
                nch = (CD[s - 1] + 127) // 128
                for j in range(nch):
                    nc.tensor.matmul(pc[0:32, 0:1], WH(f"cvc{s}_{j}"),
                                     SB(f"cx{s}_{j}"), start=(j == 0),
                                     stop=(j == nch - 1))
                cp = work.tile([32, 1], F32, tag=f"cpart{s}", name=f"cpart{s}")
                nc.vector.tensor_scalar(cp[:], pc[0:32, 0:1], S(f"cb{s}"), None,
                                        OP.add)
                cpart[s] = cp

            # ---- sequential chain with one-stage-ahead PE emission ----
            P, R = {}, {}
            O = pout.tile([80, 1], F32, tag="out", name="Oout")

            for s in range(1, 6):
                if s >= 2:
                    # eager: stage-s PE tiles whose x-chunks are complete
                    P[s] = pgi.tile([96, 3], F32, tag="gi", name=f"Pgi{s}")
                    chunks = _gi_chunks(s)
                    first = True
                    for j in (0, 2, 1):
                        for (c, rows) in chunks[:-1]:
                            mm(P[s][:, j:j + 1], f"gi{s}_{c}_{j}", c, rows,
                               start=first, stop=False)
                            first = False
                    R[s] = pconv.tile([32, 1], F32, tag="cv", name=f"R{s}")
                    cchunks = _cvx_chunks(s)
                    first = True
                    for (c, rows) in cchunks[:-2]:
                        mm(R[s][:], f"cvx{s}_{c}", c, rows,
                           start=first, stop=False)
                        first = False
                    c, rows = _OUT_CHUNKS[s - 2]
                    mm(O[:], f"out{c}", c, rows, start=(s == 2), stop=False)
                else:
                    P[1] = pgi.tile([96, 3], F32, tag="gi", name="Pgi1")
                    R[1] = pconv.tile([32, 1], F32, tag="cv", name="R1")

                Ps, Rs = P[s], R[s]
                c_last, rows_last = _gi_chunks(s)[-1]
                for ji, j in enumerate((0, 2, 1)):
                    mm(Ps[:, j:j + 1], f"gi{s}_{c_last}_{j}", c_last, rows_last,
                       start=(s == 1 and ji == 0), stop=(ji == 2))

                r = work.tile([96, 1], F32, tag="r", name=f"r{s}")
                z = work.tile([96, 1], F32, tag="zz", name=f"z{s}")
                nc.scalar.activation(r[:], Ps[:, 0:1], AF.Sigmoid,
                                     bias=ghsb[s][:, 0:1])
                nc.scalar.activation(z[:], Ps[:, 1:2], AF.Sigmoid,
                                     bias=ghsb[s][:, 1:2])
                t2 = work.tile([96, 1], F32, tag="t2", name=f"t2_{s}")
                # t2 = gh_n * r + gi_n
                nc.vector.scalar_tensor_tensor(t2[:], ghsb[s][:, 2:3], r[:],
                                               Ps[:, 2:3], OP.mult, OP.add)
                c_ = work.tile([96, 1], F32, tag="c_", name=f"c{s}_")
                nc.scalar.activation(c_[:], t2[:], AF.Tanh, bias=S(f"bin{s}"))
                cn = work.tile([96, 1], F32, tag="cn", name=f"cn{s}")
                nc.vector.tensor_add(cn[:], c_[:], S(f"nh{s}"))
                hnp = work.tile([96, 1], F32, tag="hnp", name=f"hnp{s}")
                # hnp = (h - c) * z
                nc.vector.scalar_tensor_tensor(hnp[:], S(f"hc{s}"), c_[:], z[:],
                                               OP.subtract, OP.mult)
                t3 = work.tile([96, 1], F32, tag="t3", name=f"t3_{s}")
                nc.vector.tensor_scalar(t3[:], hnp[:], cn[:], -1.0, OP.add, OP.max)
                hn3 = work.tile([96, 1], F32, tag="hn3", name=f"hn3_{s}")
                nc.vector.tensor_scalar(hn3[:], t3[:], 1.0, None, OP.min)
                hnh = work.tile([96, 1], BF16, tag="hnh", name=f"hnh{s}")
                nc.vector.tensor_copy(hnh[:], hn3[:])

                Q = pglu.tile([96, 1], F32, tag="glu", name=f"Q{s}")
                nc.tensor.matmul(Q[:], WH(f"glu{s}"), hnh[:], start=True,
                                 stop=True)
                sg = work.tile([96, 1], F32, tag="sg", name=f"sg{s}")
                nc.scalar.activation(sg[:], Q[:], AF.Sigmoid)
                g0 = work.tile([96, 1], F32, tag="g0", name=f"g0_{s}")
                nc.vector.scalar_tensor_tensor(g0[:], sg[:], hn3[:], S(f"ng{s}"),
                                               OP.mult, OP.add)
                gf = work.tile([96, 1], F32, tag="gf", name=f"gf{s}")
                nc.vector.tensor_scalar(gf[:], g0[:], -1.0, 1.0, OP.max, OP.min)
                nc.vector.tensor_copy(XH[0:96, s:s + 1], gf[:])

                cchunks = _cvx_chunks(s)
                for idx, (c, rows) in enumerate(cchunks[-2:]):
                    mm(Rs[:], f"cvx{s}_{c}", c, rows,
                       start=(s == 1 and idx == 0), stop=(idx == 1))
                cv = work.tile([32, 1], F32, tag="cv_", name=f"cv{s}_")
                nc.scalar.activation(cv[:], Rs[:], AF.Tanh, bias=cpart[s][:])
                cv0 = work.tile([32, 1], F32, tag="cv0", name=f"cv0_{s}")
                nc.vector.tensor_scalar(cv0[:], cv[:], S(f"ncv{s}"), -1.0,
                                        OP.add, OP.max)
                cvf = work.tile([32, 1], F32, tag="cvf", name=f"cvf{s}")
                nc.vector.tensor_scalar(cvf[:], cv0[:], 1.0, None, OP.min)
                # split at base partition 0, then move (tensor_scalar permits
                # differing start partitions; tensor_tensor does not)
                cvh0 = work.tile([32, 1], BF16, tag="cvh0", name=f"cvh0_{s}")
                nc.vector.tensor_copy(cvh0[:], cvf[:])
                nc.vector.tensor_scalar_add(XH[96:128, s - 1:s], cvh0[:], 0.0)

            # ---- output projection tail (chunks 4,5 need stage-5 outputs)
            for (c, rows) in _OUT_CHUNKS[4:]:
                mm(O[:], f"out{c}", c, rows, start=False, stop=(c == 5))
            y_sb = work.tile([80, 1], F32, tag="y", name="y_sb")
            nc.vector.tensor_copy(y_sb[:], O[:])
            nc.sync.dma_start(out=ydram[:], in_=y_sb[:])

    nc.compile()
    return nc


_NC_CACHE = None


def _get_nc():
    global _NC_CACHE
    if _NC_CACHE is None:
        _NC_CACHE = _build_nc()
    return _NC_CACHE


def kernel(**inputs) -> np.ndarray:
    from concourse.bass_utils import run_bass_kernel_spmd

    nc = _get_nc()
    in_map = _pack(inputs)
    in_maps = [in_map for _ in range(N_CORES)]
    res = run_bass_kernel_spmd(nc, in_maps, list(range(N_CORES)))
    y = np.asarray(res.results[0]["y"]).reshape(-1)
    return y.reshape(1, 4, 20).astype(np.float32)


# revision 23
# speedup vs baseline: 1.7676x; 1.0514x over previous
"""Trainium2 Bass kernel for nn_CoreDecoderStatefull (single-step stateful decoder).

Structure: dense -> 5x [GRU cell -> GLU -> concat -> stateful conv1d(k=2) -> concat]
-> output projection.  batch=1, seq=1: every matmul is a vector-matrix product.

HW cost on TRN2 for this kernel is dominated by per-instruction overhead and
cross-engine semaphore hops on the serial recurrence, so the design minimizes
instructions on the critical chain:

  * Weights are single-copy bf16 (correctness gate 2e-2; measured ~4e-3).
    Every matmul is W.T-stationary with an x-column as the moving operand.
  * gh_r/gh_z accumulate into the same PSUM bank as gi -> ONE sigmoid ACT
    over [96,2] yields r and u=1-z (z-gate weights negated host-side;
    sigmoid(-x) = 1-sigmoid(x)).
  * The GRU tail runs almost entirely on the ACT engine (in-order, no
    cross-engine sems): c = tanh(r*gh_n + (gi_n+bi_n)) and
    e = u*(c-h) + (h+noise) are single activations with per-partition
    scale/bias APs; the DVE computes both bias vectors in parallel.
  * Noise-adds at tanh-bounded sites (x0, GLU out, conv out) skip the
    reference's +-1 clamp: tanh output can exceed 1 only by the noise
    amplitude 1/254, a <=4e-3 absolute, sub-gate effect on rare saturated
    elements.  The GRU state hn keeps its clamp (h is unbounded).
  * All chain outputs are written bf16 directly into the [128,6] concat tile
    XH (chunk c rows 0:96 = x0|g_c, rows 96:128 = conv out cv_{c+1}); weight
    rows are permuted host-side to match.
  * PSUM: one start=True on the first matmul into a bank (gh_r for the P
    banks), one stop=True on the last; in-between matmuls start=False.
  * PE is in-order, so eager work (next stage's gh + gi/cvx chunks whose
    x-chunks are complete, out-projection chunks, conv c-taps) is emitted in
    the PE stream right AFTER the current stage's serial matmuls, filling PE
    idle slots under the ACT/DVE chain without delaying it.
  * Weight DMAs are dispatched from 4 engine queues (a single queue pays
    ~565ns SP sequencer time per DMA, serializing arrival) in need order:
    head slab (dense+gh1) and stage-1 slab first.
  * Noise sites are deterministic (jax fold_in(key(42), i)) -> precomputed.
"""

import numpy as np
from contextlib import ExitStack

GD = [96, 224, 352, 480, 608]   # GRU input dims per stage
CD = [192, 320, 448, 576, 704]  # conv input dims per stage
N_CORES = 8
N_SLABS = 8


# ---------------------------------------------------------------------------
# reference x-vector index mapping
# ---------------------------------------------------------------------------
def _refidx(c: int, r: int) -> int:
    if r < 96:
        return r if c == 0 else GD[c - 1] + r      # x0 or g_c
    assert c <= 4
    return CD[c] + (r - 96)                        # cv_{c+1}


def _gi_chunks(s):
    return [(c, 128) for c in range(s - 1)] + [(s - 1, 96)]


def _cvx_chunks(s):
    return [(c, 128) for c in range(s - 1)] + [(s - 1, 96), (s, 96)]


_OUT_CHUNKS = [(c, 128) for c in range(5)] + [(5, 97)]  # row 96 = 1.0 (b_out)


# ---------------------------------------------------------------------------
# static layout
# ---------------------------------------------------------------------------
def _layout():
    # bf16 PE-weight slabs: 7 = head (dense + gh1), 0 = conv c-taps,
    # 1..5 = stage chains (stage-s recurrent weights + gh for stage s+1),
    # 6 = output projection.
    wt = {}
    slab_cols = [0] * N_SLABS

    def put(name, slab, rows, ncols):
        wt[name] = (slab, slab_cols[slab], rows, ncols)
        slab_cols[slab] += ncols

    put("dense", 7, 81, 96)
    for j in range(3):
        put(f"gh1_{j}", 7, 97, 96)
    for s in range(1, 6):
        cd = CD[s - 1]
        nch = (cd + 127) // 128
        for j in range(nch):
            rows = min(128, cd - 128 * j)
            # last chunk gets an aug-1.0 row carrying the conv bias
            put(f"cvc{s}_{j}", 0, rows + (1 if j == nch - 1 else 0), 32)

    for s in range(1, 6):
        for (c, rows) in _gi_chunks(s):
            for j in range(3):
                put(f"gi{s}_{c}_{j}", s, rows, 96)
        put(f"glu{s}", s, 96, 96)
        for (c, rows) in _cvx_chunks(s):
            put(f"cvx{s}_{c}", s, rows, 32)
        if s < 5:
            for j in range(3):
                put(f"gh{s + 1}_{j}", s, 97, 96)
    for (c, rows) in _OUT_CHUNKS:
        put(f"out{c}", 6, rows, 80)

    # stile: fp32 column vectors
    st = {}
    scol = 0

    def sput(name, rows):
        nonlocal scol
        st[name] = (scol, rows)
        scol += 1

    sput("n0", 96)
    for s in range(1, 6):
        sput(f"hn{s}", 96)             # h + output-noise (GRU blend offset)
        sput(f"nhc{s}", 96)            # -h (for u*(c-h) via u*c + u*(-h))
        sput(f"ng{s}", 96)             # GLU noise
        sput(f"ncv{s}", 32)            # conv noise
        sput(f"bin{s}", 96)            # bi[192:288] (gi n-gate bias)

    # bf16 input columns (matmul operands must be bf16);
    # aug 1.0 rows fold the biases: z row 80 = 1.0, h row 96 = 1.0
    sb = {}
    bcol = 0

    def bput(name, rows):
        nonlocal bcol
        sb[name] = (bcol, rows)
        bcol += 1

    bput("zx", 81)
    for s in range(1, 6):
        bput(f"hx{s}", 97)
    for s in range(1, 6):
        cd = CD[s - 1]
        nch = (cd + 127) // 128
        for j in range(nch):
            rows = min(128, cd - 128 * j)
            bput(f"cx{s}_{j}", rows + (1 if j == nch - 1 else 0))
    return wt, slab_cols, st, scol, sb, bcol


_WT, _SLAB_COLS, _ST, _ST_COLS, _SB, _SB_COLS = _layout()


# ---------------------------------------------------------------------------
# host-side packing
# ---------------------------------------------------------------------------
def _noise_vectors():
    import jax
    import jax.numpy as jnp

    vs = {}
    for i in range(16):
        n = 96 if (i == 0 or i % 3 != 0) else 32
        u = jax.random.uniform(
            jax.random.fold_in(jax.random.key(42), i), (1, n), dtype=jnp.float32
        )
        vs[i] = (np.asarray(u).reshape(-1) - 0.5) / np.float32(127.0)
    return vs


def _to_ml_bf16(a):
    import ml_dtypes

    return np.asarray(a, np.float32).astype(ml_dtypes.bfloat16)


def _pack(inp):
    f32 = np.float32
    slabs = [np.zeros((128, c), f32) for c in _SLAB_COLS]
    stile = np.zeros((128, _ST_COLS), f32)

    def wfill(name, block):
        slab, col, rows, ncols = _WT[name]
        assert block.shape == (rows, ncols), (name, block.shape)
        slabs[slab][:rows, col:col + ncols] = block

    def sfill(name, vec):
        col, rows = _ST[name]
        assert vec.shape == (rows,), (name, vec.shape)
        stile[:rows, col] = vec

    sbf = np.zeros((128, _SB_COLS), f32)

    def bfill(name, vec):
        col, rows = _SB[name]
        assert vec.shape == (rows,), (name, vec.shape)
        sbf[:rows, col] = vec

    # dense + gh (bias folded via aug-1 rows; z gate negated -> ACT gives 1-z)
    blk = np.zeros((81, 96), f32)
    blk[:80] = inp["w_dense"].T
    blk[80] = inp["b_dense"]
    wfill("dense", blk)
    for s in range(1, 6):
        wh, bi, bh = inp[f"g{s}_wh"], inp[f"g{s}_bi"], inp[f"g{s}_bh"]
        for j in range(3):
            sgn = -1.0 if j == 1 else 1.0
            blk = np.zeros((97, 96), f32)
            blk[:96] = sgn * wh[96 * j:96 * (j + 1), :].T
            blk[96] = sgn * ((bi + bh)[96 * j:96 * (j + 1)] if j < 2
                             else bh[192:288])
            wfill(f"gh{s}_{j}", blk)
        cw0 = inp[f"cv{s}_w"][:, :, 0].astype(f32)
        cd = CD[s - 1]
        nch = (cd + 127) // 128
        for j in range(nch):
            rows = min(128, cd - 128 * j)
            blk = cw0[:, 128 * j:128 * j + rows].T
            if j == nch - 1:
                blk = np.vstack([blk, inp[f"cv{s}_b"].astype(f32)[None]])
            wfill(f"cvc{s}_{j}", blk)

    # recurrent-path PE weights (gi z gate negated to match)
    for s in range(1, 6):
        wi = inp[f"g{s}_wi"].astype(f32)
        for (c, rows) in _gi_chunks(s):
            ridx = [_refidx(c, r) for r in range(rows)]
            for j in range(3):
                sgn = -1.0 if j == 1 else 1.0
                wfill(f"gi{s}_{c}_{j}", sgn * wi[96 * j:96 * (j + 1), ridx].T)
        wfill(f"glu{s}", inp[f"glu{s}_w"].T.astype(f32))
        cw1 = inp[f"cv{s}_w"][:, :, 1].astype(f32)
        for (c, rows) in _cvx_chunks(s):
            ridx = [_refidx(c, r) for r in range(rows)]
            wfill(f"cvx{s}_{c}", cw1[:, ridx].T)
    w_out, b_out = inp["w_out"].astype(f32), inp["b_out"].astype(f32)
    for (c, rows) in _OUT_CHUNKS:
        if c < 5:
            ridx = [_refidx(c, r) for r in range(rows)]
            wfill(f"out{c}", w_out[:, ridx].T)
        else:
            blk = np.zeros((97, 80), f32)
            blk[:96] = w_out[:, 608:704].T
            blk[96] = b_out
            wfill(f"out{c}", blk)

    # state tile
    nv = _noise_vectors()
    sfill("n0", nv[0])
    for s in range(1, 6):
        h = inp[f"h{s}"].reshape(-1).astype(f32)
        sfill(f"hn{s}", h + nv[3 * s - 2])
        sfill(f"nhc{s}", -h)
        sfill(f"ng{s}", nv[3 * s - 1])
        sfill(f"ncv{s}", nv[3 * s])
        sfill(f"bin{s}", inp[f"g{s}_bi"][192:288].astype(f32))

    # bf16 input columns for the gh/cvc/dense matmuls (aug row = 1.0)
    zv = np.zeros(81, f32)
    zv[:80] = inp["z"].reshape(-1)
    zv[80] = 1.0
    bfill("zx", zv)
    for s in range(1, 6):
        hv = np.zeros(97, f32)
        hv[:96] = inp[f"h{s}"].reshape(-1)
        hv[96] = 1.0
        bfill(f"hx{s}", hv)
        cv = inp[f"c{s}"].reshape(-1).astype(f32)
        cd = CD[s - 1]
        nch = (cd + 127) // 128
        for j in range(nch):
            rows = min(128, cd - 128 * j)
            seg = cv[128 * j:128 * j + rows]
            if j == nch - 1:
                seg = np.concatenate([seg, np.ones(1, f32)])
            bfill(f"cx{s}_{j}", seg)

    m = {f"wslab{i}": _to_ml_bf16(slabs[i]) for i in range(N_SLABS)}
    m["stile"] = stile
    m["sbf16"] = _to_ml_bf16(sbf)
    return m


# ---------------------------------------------------------------------------
# device program
# ---------------------------------------------------------------------------
def _build_nc(loop_iters=None, dma_only=False, compute_only=False, empty=False,
              debug_taps=False):
    from concourse import bacc, tile, mybir

    F32 = mybir.dt.float32
    BF16 = mybir.dt.bfloat16
    AF = mybir.ActivationFunctionType
    OP = mybir.AluOpType

    nc = bacc.Bacc("TRN2", target_bir_lowering=False, debug=False,
                   num_devices=N_CORES)
    wdram = {i: nc.dram_tensor(f"wslab{i}", [128, _SLAB_COLS[i]], BF16,
                               kind="ExternalInput") for i in range(N_SLABS)}
    sdram = nc.dram_tensor("stile", [128, _ST_COLS], F32, kind="ExternalInput")
    bdram = nc.dram_tensor("sbf16", [128, _SB_COLS], BF16, kind="ExternalInput")
    ydram = nc.dram_tensor("y", [80, 1], F32, kind="ExternalOutput")
    dbg = {}
    if debug_taps:
        for nm, rows in [("x0", 96), ("ru1", 96), ("ghn1", 96), ("c1", 96),
                         ("hnh1", 96), ("g1", 96), ("cv1", 32), ("ru2", 96),
                         ("hnh2", 96), ("P1r", 96), ("P1n", 96), ("be1", 96),
                         ("cp1", 32), ("R1s", 32), ("cvt1", 32)]:
            dbg[nm] = nc.dram_tensor(f"dbg_{nm}", [rows, 2], F32,
                                     kind="ExternalOutput")

    with tile.TileContext(nc) as tc, ExitStack() as ctx:
        wpool = ctx.enter_context(tc.tile_pool(name="wpool", bufs=1))
        spool = ctx.enter_context(tc.tile_pool(name="spool", bufs=1))
        work = ctx.enter_context(tc.tile_pool(name="work", bufs=1))
        pearly = ctx.enter_context(tc.tile_pool(name="pearly", bufs=2, space="PSUM"))
        pgi = ctx.enter_context(tc.tile_pool(name="pgi", bufs=2, space="PSUM"))
        pglu = ctx.enter_context(tc.tile_pool(name="pglu", bufs=1, space="PSUM"))
        pconv = ctx.enter_context(tc.tile_pool(name="pconv", bufs=2, space="PSUM"))
        pout = ctx.enter_context(tc.tile_pool(name="pout", bufs=1, space="PSUM"))

        if loop_iters is not None:
            ctx.enter_context(tc.For_i(0, loop_iters, 1))

        if empty:
            y_sb = work.tile([80, 1], F32, tag="y", name="y_sb")
            nc.vector.memset(y_sb[:], 0.0)
            nc.sync.dma_start(out=ydram[:], in_=y_sb[:])
            nc.compile()
            return nc

        # ACT table prefetch (sigmoid_and_others holds sigmoid+tanh+identity)
        warm = work.tile([1, 1], F32, tag="warm", name="warm")
        nc.vector.memset(warm[:], 0.0)
        warm2 = work.tile([1, 1], F32, tag="warm2", name="warm2")
        nc.scalar.activation(warm2[:], warm[:], AF.Sigmoid)
        nc.scalar.activation(warm2[:], warm2[:], AF.Tanh)

        XH = work.tile([128, 6], BF16, tag="XH", name="XH")
        nc.vector.memset(XH[96:97, 5:6], 1.0)

        stile = spool.tile([128, _ST_COLS], F32, tag="stile", name="stile")
        sbf = spool.tile([128, _SB_COLS], BF16, tag="sbf", name="sbf")
        wt = {}
        for i in range(N_SLABS):
            wt[i] = wpool.tile([128, _SLAB_COLS[i]], BF16, tag=f"w{i}",
                               name=f"wt{i}")
        if not compute_only:
            # 2 dispatch queues (SP + ACT HWDGE), need order: head+stage1
            # weights first so the recurrence chain starts early
            nc.sync.dma_start(out=sbf[:], in_=bdram[:])
            nc.scalar.dma_start(out=stile[:], in_=sdram[:])
            nc.sync.dma_start(out=wt[7][:], in_=wdram[7][:])
            nc.scalar.dma_start(out=wt[2][:], in_=wdram[2][:])
            nc.sync.dma_start(out=wt[1][:], in_=wdram[1][:])
            nc.scalar.dma_start(out=wt[4][:], in_=wdram[4][:])
            nc.sync.dma_start(out=wt[3][:], in_=wdram[3][:])
            nc.scalar.dma_start(out=wt[0][:], in_=wdram[0][:])
            nc.sync.dma_start(out=wt[6][:], in_=wdram[6][:])
            nc.scalar.dma_start(out=wt[5][:], in_=wdram[5][:])
        else:
            nc.vector.memset(stile[:, 0:1], 0.01)
            nc.vector.memset(sbf[:, 0:1], 0.01)
            for i in range(N_SLABS):
                nc.vector.memset(wt[i][:, 0:1], 0.01)

        def WH(name):
            slab, col, rows, ncols = _WT[name]
            return wt[slab][0:rows, col:col + ncols]

        def S(name, rows=None):
            col, r = _ST[name]
            if rows is not None:
                r = rows
            return stile[0:r, col:col + 1]

        def SB(name):
            col, r = _SB[name]
            return sbf[0:r, col:col + 1]

        def mm(psum_ap, name, xc, rows, start, stop):
            nc.tensor.matmul(psum_ap, WH(name), XH[0:rows, xc:xc + 1],
                             start=start, stop=stop)

        if not dma_only:
            # ---- head: dense -> x0 (+ gh1 under the x0 ACT shadow) ----
            pd = pearly.tile([96, 1], F32, tag="early", name="pdense")
            nc.tensor.matmul(pd[:], WH("dense"), SB("zx"), start=True, stop=True)
            x0t = work.tile([96, 1], F32, tag="x0t", name="x0t")
            nc.scalar.activation(x0t[:], pd[:], AF.Tanh)
            nc.scalar.activation(XH[0:96, 0:1], x0t[:], AF.Identity,
                                 bias=S("n0"))

            P, R, ghn, cpart = {}, {}, {}, {}
            O = pout.tile([80, 1], F32, tag="out", name="Oout")

            def emit_gh(s):
                # gh_r opens P[s]'s bank (the ONLY start=True for it); gh_n
                # goes to its own psum then SBUF (gpsimd copy, off-path).
                for j in (0, 1):
                    nc.tensor.matmul(P[s][:96, j:j + 1], WH(f"gh{s}_{j}"),
                                     SB(f"hx{s}"), start=(j == 0), stop=False)
                pn = pearly.tile([96, 1], F32, tag="early", name=f"pghn{s}")
                nc.tensor.matmul(pn[:], WH(f"gh{s}_2"), SB(f"hx{s}"),
                                 start=True, stop=True)
                g = work.tile([96, 1], F32, tag=f"ghn{s}", name=f"ghn{s}")
                nc.vector.tensor_copy(g[:], pn[:])
                ghn[s] = g

            def emit_cvc(s):
                # conv c-state tap: input-only, feeds stage-s conv via cpart
                pc = pearly.tile([32, 1], F32, tag="early", name=f"pcvc{s}")
                nch = (CD[s - 1] + 127) // 128
                for j in range(nch):
                    nc.tensor.matmul(pc[:], WH(f"cvc{s}_{j}"), SB(f"cx{s}_{j}"),
                                     start=(j == 0), stop=(j == nch - 1))
                cp = work.tile([32, 1], F32, tag=f"cpart{s}", name=f"cpart{s}")
                nc.vector.tensor_copy(cp[:], pc[:])
                cpart[s] = cp

            P[1] = pgi.tile([96, 3], F32, tag="gi", name="Pgi1")
            R[1] = pconv.tile([32, 1], F32, tag="cv", name="R1")
            emit_gh(1)

            for s in range(1, 6):
                Ps, Rs = P[s], R[s]
                # serial-path matmuls first (PE is in-order): leftover gi
                # chunk c=s-2 (rows 96:128 = cv_{s-1}, ready only after stage
                # s-1's conv), then gi-last; r/z gates before n so the ru ACT
                # can start 2 matmuls early.  cvx leftover goes after (conv
                # isn't needed until the chain tail).
                c_last, rows_last = _gi_chunks(s)[-1]
                if s >= 2:
                    cl, rl = _gi_chunks(s)[-2]
                    for j in (0, 1):
                        mm(Ps[:, j:j + 1], f"gi{s}_{cl}_{j}", cl, rl,
                           start=False, stop=False)
                for j in (0, 1):
                    mm(Ps[:, j:j + 1], f"gi{s}_{c_last}_{j}", c_last, rows_last,
                       start=False, stop=False)
                if s >= 2:
                    mm(Ps[:, 2:3], f"gi{s}_{cl}_{2}", cl, rl,
                       start=False, stop=False)
                mm(Ps[:, 2:3], f"gi{s}_{c_last}_{2}", c_last, rows_last,
                   start=False, stop=True)
                if s >= 2:
                    cc, rc = _cvx_chunks(s)[-3]
                    mm(Rs[:], f"cvx{s}_{cc}", cc, rc, start=(s == 2),
                       stop=False)

                # r, u = sigmoid(P[:,0:2]) in ONE activation (u = 1-z)
                ru = work.tile([96, 2], F32, tag="ru", name=f"ru{s}")
                nc.scalar.activation(ru[:], Ps[:, 0:2], AF.Sigmoid)
                # DVE precomputes both ACT bias vectors, off the serial path
                ginb = work.tile([96, 1], F32, tag="ginb", name=f"ginb{s}")
                nc.vector.tensor_scalar(ginb[:], Ps[:, 2:3], S(f"bin{s}"),
                                        None, OP.add)
                be = work.tile([96, 1], F32, tag="be", name=f"be{s}")
                nc.vector.scalar_tensor_tensor(be[:], ru[:, 1:2], S(f"nhc{s}"),
                                               S(f"hn{s}"), OP.mult, OP.add)
                # c = tanh(r*gh_n + gi_n + bi_n); e = u*(c-h) + (h+noise)
                c_ = work.tile([96, 1], F32, tag="c_", name=f"c{s}_")
                nc.scalar.activation(c_[:], ru[:, 0:1], AF.Tanh,
                                     scale=ghn[s][:], bias=ginb[:])
                e = work.tile([96, 1], F32, tag="e", name=f"e{s}")
                nc.scalar.activation(e[:], c_[:], AF.Identity,
                                     scale=ru[:, 1:2], bias=be[:])
                hnh = work.tile([96, 1], BF16, tag="hnh", name=f"hnh{s}")
                nc.vector.tensor_scalar(hnh[:], e[:], -1.0, 1.0, OP.max, OP.min)

                if debug_taps and s <= 2:
                    def tap(nm, ap, rows, cols=1, cast=False):
                        if cast:
                            t = work.tile([rows, cols], F32, tag=f"dbg{nm}",
                                          name=f"dbg{nm}")
                            nc.vector.tensor_copy(t[:], ap)
                            ap = t[:]
                        nc.sync.dma_start(out=dbg[nm][0:rows, 0:cols], in_=ap)
                    tap(f"ru{s}", ru[:], 96, 2)
                    if s == 1:
                        tap("x0", XH[0:96, 0:1], 96, cast=True)
                        tap("ghn1", ghn[1][:], 96)
                        tap("c1", c_[:], 96)
                        tap("be1", be[:], 96)
                        tap("P1r", Ps[:, 0:1], 96, cast=True)
                        tap("P1n", Ps[:, 2:3], 96, cast=True)
                    tap(f"hnh{s}", hnh[:], 96, cast=True)

                # PE slack under the ACT chain: eager work for later stages
                if s == 1:
                    emit_cvc(1)
                if s < 5:
                    P[s + 1] = pgi.tile([96, 3], F32, tag="gi",
                                        name=f"Pgi{s + 1}")
                    R[s + 1] = pconv.tile([32, 1], F32, tag="cv",
                                          name=f"R{s + 1}")
                    emit_gh(s + 1)
                    chunks = _gi_chunks(s + 1)
                    for j in (0, 1, 2):
                        for (c, rows) in chunks[:-2]:
                            mm(P[s + 1][:, j:j + 1], f"gi{s + 1}_{c}_{j}", c,
                               rows, start=False, stop=False)
                    for ci, (c, rows) in enumerate(_cvx_chunks(s + 1)[:-3]):
                        mm(R[s + 1][:], f"cvx{s + 1}_{c}", c, rows,
                           start=(ci == 0), stop=False)
                if s == 1:
                    for t in range(2, 6):
                        emit_cvc(t)
                if s >= 2:
                    c, rows = _OUT_CHUNKS[s - 2]
                    mm(O[:], f"out{c}", c, rows, start=(s == 2), stop=False)

                # GLU: g = hn*sigmoid(glu_w@hn) + noise, bf16 into XH
                Q = pglu.tile([96, 1], F32, tag="glu", name=f"Q{s}")
                nc.tensor.matmul(Q[:], WH(f"glu{s}"), hnh[:], start=True,
                                 stop=True)
                sg = work.tile([96, 1], F32, tag="sg", name=f"sg{s}")
                nc.scalar.activation(sg[:], Q[:], AF.Sigmoid)
                nc.scalar.activation(XH[0:96, s:s + 1], hnh[:], AF.Identity,
                                     scale=sg[:], bias=S(f"ng{s}"))

                # conv: cv = tanh(x-taps + c-tap) + noise, bf16 into XH
                cchunks = _cvx_chunks(s)
                for idx, (c, rows) in enumerate(cchunks[-2:]):
                    mm(Rs[:], f"cvx{s}_{c}", c, rows,
                       start=(s == 1 and idx == 0), stop=(idx == 1))
                cv = work.tile([32, 1], F32, tag="cv_", name=f"cv{s}_")
                nc.scalar.activation(cv[:], Rs[:], AF.Tanh, bias=cpart[s][:])
                cvn = work.tile([32, 1], BF16, tag="cvn", name=f"cvn{s}")
                nc.vector.tensor_scalar(cvn[:], cv[:], S(f"ncv{s}"), None,
                                        OP.add)
                # tensor_scalar permits differing in/out start partitions
                nc.vector.tensor_scalar_add(XH[96:128, s - 1:s], cvn[:], 0.0)
                if debug_taps and s == 1:
                    tg = work.tile([96, 1], F32, tag="dbgg1", name="dbgg1")
                    nc.vector.tensor_copy(tg[:], XH[0:96, 1:2])
                    nc.sync.dma_start(out=dbg["g1"][0:96, 0:1], in_=tg[:])
                    tcv = work.tile([32, 1], F32, tag="dbgcv1", name="dbgcv1")
                    nc.vector.tensor_copy(tcv[:], XH[96:128, 0:1])
                    nc.sync.dma_start(out=dbg["cv1"][0:32, 0:1], in_=tcv[:])
                    nc.sync.dma_start(out=dbg["cp1"][0:32, 0:1],
                                      in_=cpart[1][:])
                    tr1 = work.tile([32, 1], F32, tag="dbgR1", name="dbgR1")
                    nc.vector.tensor_copy(tr1[:], Rs[:])
                    nc.sync.dma_start(out=dbg["R1s"][0:32, 0:1], in_=tr1[:])
                    nc.sync.dma_start(out=dbg["cvt1"][0:32, 0:1], in_=cv[:])

            # ---- output projection tail (chunks 4,5 need stage-5 outputs)
            for (c, rows) in _OUT_CHUNKS[4:]:
                mm(O[:], f"out{c}", c, rows, start=False, stop=(c == 5))
            y_sb = work.tile([80, 1], F32, tag="y", name="y_sb")
            nc.vector.tensor_copy(y_sb[:], O[:])
            nc.sync.dma_start(out=ydram[:], in_=y_sb[:])

    nc.compile()
    return nc


_NC_CACHE = None


def _get_nc():
    global _NC_CACHE
    if _NC_CACHE is None:
        _NC_CACHE = _build_nc()
    return _NC_CACHE


def kernel(**inputs) -> np.ndarray:
    from concourse.bass_utils import run_bass_kernel_spmd

    nc = _get_nc()
    in_map = _pack(inputs)
    in_maps = [in_map for _ in range(N_CORES)]
    res = run_bass_kernel_spmd(nc, in_maps, list(range(N_CORES)))
    y = np.asarray(res.results[0]["y"]).reshape(-1)
    return y.reshape(1, 4, 20).astype(np.float32)
